# revision 14
# baseline (speedup 1.0000x reference)
"""Llama attention layer (B=2, S=2048, D=2048, H=16, HD=128, RoPE, causal)
on 8 Trainium2 NeuronCores.

Sharding: core c -> (batch b = c//4, head group g = c%4 of 4 heads).
Each core computes q/k/v projections for its 512 columns of wq/wk/wv,
RoPE, causal attention for its 4 heads, and the out-projection against
its 512 rows of wo (a partial sum over head groups). The host sums the
4 partials per batch and stacks the 2 batches.

All device matmuls run in bf16 with fp32 PSUM accumulation. Softmax is
computed without max-subtraction (scores here are bounded ~|9|), with
the denominator obtained from an M=1 ones-matmul over exp(scores^T).

Perf structure (vs the naive version; ~353us -> ~326us):
- xT is packed m-major on the host AND in SBUF so every DMA line is
  contiguous (strided 256B-line SBUF writes run ~7x slower); the
  K/Q-projection matmuls read it through a strided [128,4,128] rhs AP.
- Input DMAs are split over both hardware DGE queues (SP + Activation)
  because descriptor generation costs ~0.6us of sequencer time each.
- The V-projection runs k-outer over waves of 4 PSUM banks so each wv
  k-slice unlocks matmuls while the DMA trickles in; a few warmup
  matmuls on zeros pin the PE p-state early (the PE only reaches full
  clock after ~3us of continuous execution).
- Diagonal 128x512 score/exp/attn-V work is narrowed to the causally
  valid query columns; the V-matmul is split per 128-col region so each
  region's accumulation group can close with its own stop flag.
- Softmax denominator (phases 2+3): DVE accumulates all exp-chunk
  pairs into one running total; a SINGLE ones-matmul per (qt, h) does
  the partition reduction, deferred into the next attend's chunk loop
  so the in-order PE never waits on the DVE chain (PE sumexp work
  drops from qt+2 matmuls per (qt,h) to 1). Phase-1 tail attends keep
  the self-contained immediate-pair scheme (only 4 PSUM banks free).
- Output rows are staged in [128, 2048] tiles: ONE output DMA per
  query-row block instead of four (descriptor gen is ~0.6us each on
  the SP sequencer); casts alternate DVE/ACT; the final row block
  DMAs per-slice so the tail transfer starts ASAP.
- The warm-exp (ACT Exp-table load, ~1.3us) is issued after the
  Activation-queue DMA descriptors, not before, so the weight stream
  starts ~1.3us earlier; wv k=0 rides the SP queue ahead of xT.
- Attention for (qt=0, heads 0-1) runs at the tail of phase 1 on the 4
  free PSUM banks, hiding the phase-1 PSUM drain / phase-2 pool-open
  latency; each q-tile's out-projection is interleaved between the next
  tile's heads to hide the normalize latency.
- Output partials are written bf16 (host accumulates in fp32), halving
  output DMA.
"""

import os
import sys

import numpy as np
import ml_dtypes

if "/opt/trn_rl_repo" not in sys.path:
    sys.path.insert(0, "/opt/trn_rl_repo")

import concourse.bass as bass  # noqa: E402
import concourse.mybir as mybir  # noqa: E402
import concourse.bacc as bacc  # noqa: E402
import concourse.tile as tile  # noqa: E402

BF16 = ml_dtypes.bfloat16

B, S, D, H = 2, 2048, 2048, 16
HD = D // H            # 128, head dim
G = 4                  # head groups (cores per batch)
NH = H // G            # 4 heads per core
DG = NH * HD           # 512, per-core head width
P = 128
KO = D // P            # 16 k-subtiles over D
NKT = S // P           # 16 key chunks of 128
NQT = S // 512         # 4 q tiles of 512
QT = 512
ROPE_THETA = 10000.0
SCALE = 1.0 / float(np.sqrt(HD))

N_CORES = 8

_BUILT = None  # (nc,) cache


def build_module():
    fp32 = mybir.dt.float32
    bf16 = mybir.dt.bfloat16

    nc = bacc.Bacc("TRN2", target_bir_lowering=False, debug=False,
                   num_devices=N_CORES, num_swdge_queues=4)

    xT = nc.dram_tensor("xT", [P, NKT, KO, P], bf16, kind="ExternalInput")
    wq = nc.dram_tensor("wq", [P, KO, DG], bf16, kind="ExternalInput")
    wk = nc.dram_tensor("wk", [P, KO, DG], bf16, kind="ExternalInput")
    wv = nc.dram_tensor("wv", [P, KO, DG], bf16, kind="ExternalInput")
    wo = nc.dram_tensor("wo", [P, NH, D], bf16, kind="ExternalInput")
    cosT = nc.dram_tensor("cosT", [P, S], bf16, kind="ExternalInput")
    sinT = nc.dram_tensor("sinT", [P, S], bf16, kind="ExternalInput")
    maskT = nc.dram_tensor("maskT", [P, P], bf16, kind="ExternalInput")
    out = nc.dram_tensor("out", [P, NKT, D], bf16, kind="ExternalOutput")

    Exp = mybir.ActivationFunctionType.Exp

    with tile.TileContext(nc) as tc:
        # Pools are deliberately few: every tile_pool open/close costs an
        # all-engine barrier round (~0.5us each, and the closes stack up in
        # the program epilogue), so logical groups live as TAGS inside a
        # shared pool instead of separate pools.
        with tc.tile_pool(name="big", bufs=1) as big:
            ones = big.tile([P, P], bf16)
            nc.vector.memset(ones, 1.0)
            warm = big.tile([1, 1], fp32)

            qT_sb = big.tile([P, NH, S], bf16)   # per head: [HD, S]
            kT_sb = big.tile([P, NH, S], bf16)
            v_sb = big.tile([P, NKT, DG], bf16)  # [key%128, keychunk, dg]
            wo_sb = big.tile([P, NH, D], bf16)
            mask_sb = big.tile([P, P], bf16)
            ao0_sb = big.tile([P, NH, QT], bf16)  # qt=0 attention out

            # --- deferred-finalize attention --------------------------
            # Each attend's denominator is built entirely on DVE (pair adds
            # + a running total); the partition reduction is ONE ones-matmul
            # per (qt, h), deferred until after more independent PE work has
            # been queued so the in-order PE never waits on the DVE chain.
            pending_fin = []  # [(total, ps_o, dst)] awaiting normalize

            def flush_fin(pool_sum, pool_ep, ep_bufs):
                while pending_fin:
                    total, ps_o_p, dst_p = pending_fin.pop()
                    ps_sum = pool_sum.tile([P, QT], fp32, tag="ps_sum",
                                           bufs=1)
                    nc.tensor.matmul(ps_sum, ones, total,
                                     start=True, stop=True)
                    rec = pool_ep.tile([P, QT], fp32, tag="rec",
                                       bufs=ep_bufs)
                    nc.vector.reciprocal_approx_fast(rec, ps_sum)
                    nc.vector.tensor_mul(dst_p, ps_o_p, rec)

            def attend_scores(qt, h, m, st, pools):
                """Scores matmul + exp for key-chunk m of (qt, h)."""
                (pool_s, s_bufs, pool_o, o_bufs, pool_ax, ax_bufs,
                 pool_axp, axp_bufs, pool_tot, tot_bufs) = pools
                o = m - qt * 4
                colo = max(0, o) * P
                ps_s = pool_s.tile([P, QT], fp32, tag="ps_s", bufs=s_bufs)
                nc.tensor.matmul(ps_s[:, colo:],
                                 kT_sb[:, h, m * P:(m + 1) * P],
                                 qT_sb[:, h,
                                       qt * QT + colo:(qt + 1) * QT],
                                 start=True, stop=True)
                ax = pool_ax.tile([P, QT], bf16, tag="ax", bufs=ax_bufs)
                if colo:
                    # zero the causally-dead prefix so the denominator adds
                    # see zeros there
                    nc.gpsimd.memset(ax[:, 0:colo], 0.0)
                nc.scalar.activation(ax[:, colo:], ps_s[:, colo:],
                                     Exp, scale=SCALE)
                st[("ax", m)] = ax

            def attend_av(qt, h, m, st, pools):
                """Mask + attn@V + denominator adds for key-chunk m."""
                (pool_s, s_bufs, pool_o, o_bufs, pool_ax, ax_bufs,
                 pool_axp, axp_bufs, pool_tot, tot_bufs) = pools
                if m == 0:
                    st["ps_o"] = pool_o.tile([P, QT], fp32, tag="ps_o",
                                             bufs=o_bufs, name="ps_o")
                ps_o = st["ps_o"]
                ax = st.pop(("ax", m))
                o = m - qt * 4
                colo = max(0, o) * P
                if o >= 0:
                    # triangular mask on the 128 cols straddling the diagonal
                    nc.vector.tensor_mul(ax[:, colo:colo + P],
                                         ax[:, colo:colo + P], mask_sb)
                # attn @ V, narrowed; each 128-col region's last contribution
                # is its diagonal chunk, which carries stop=True
                vsl = v_sb[:, m, h * HD:(h + 1) * HD]
                if o < 0:
                    nc.tensor.matmul(ps_o, vsl, ax,
                                     start=(m == 0), stop=False)
                else:
                    nc.tensor.matmul(ps_o[:, colo:colo + P], vsl,
                                     ax[:, colo:colo + P],
                                     start=(m == 0), stop=True)
                    if colo + P < QT:
                        nc.tensor.matmul(ps_o[:, colo + P:], vsl,
                                         ax[:, colo + P:],
                                         start=(m == 0), stop=False)
                if m % 2 == 0:
                    st["ax_prev"] = ax
                else:
                    pair = pool_axp.tile([P, QT], bf16, tag="axp",
                                         bufs=axp_bufs)
                    nc.vector.tensor_add(pair, st["ax_prev"], ax)
                    if m == 1:
                        st["pair_first"] = pair
                    elif m == 3:
                        tot = pool_tot.tile([P, QT], bf16, tag="tot",
                                            bufs=tot_bufs, name="tot")
                        nc.vector.tensor_add(tot, st["pair_first"], pair)
                        st["total"] = tot
                    else:
                        nc.vector.tensor_add(st["total"], st["total"], pair)

            def attend_chunk(qt, h, m, st, pools):
                """One key-chunk of causal attention for (qt, h)."""
                attend_scores(qt, h, m, st, pools)
                attend_av(qt, h, m, st, pools)

            # ---------------- phase 1: projections + RoPE ----------------
            with tc.tile_pool(name="w_pool", bufs=1) as w_pool, \
                 tc.tile_pool(name="ps1", bufs=1, space="PSUM") as ps1:
                # DMA order matters: wv first (V-loop gate), then xT in
                # m-major column blocks (the dram layout is packed so block m
                # is contiguous) so V m-group m only waits for its own block,
                # then the K/Q-phase tensors, then phase-2/3 tensors.
                wv_sb = w_pool.tile([P, KO, DG], bf16)
                # m-major like the DRAM packing: per-block DMA is contiguous
                # (4KB/partition). A k-major SBUF layout would make the block
                # DMA scatter 256B lines, which runs ~7x slower.
                xT_sb = w_pool.tile([P, NKT, KO, P], bf16)
                wk_sb = w_pool.tile([P, KO, DG], bf16)
                cos_sb = w_pool.tile([P, S], bf16)
                sin_sb = w_pool.tile([P, S], bf16)
                wq_sb = w_pool.tile([P, KO, DG], bf16)
                # Descriptor generation costs ~0.6us of sequencer time per
                # dma_start, so split the input stream over BOTH hardware DGE
                # queues. wv is the V-phase gate, so it is split ACROSS both
                # queues to land as fast as possible: singles k0-3 lead the
                # SP queue (which starts pumping ~1.8us before the Activation
                # queue), groups k4-15 lead the Activation queue. xT blocks
                # follow on SP; the K/Q/phase-2 tensors follow on Activation.
                for k in range(4):
                    nc.sync.dma_start(wv_sb[:, k, :], wv.ap()[:, k, :])
                for m in range(NKT):
                    nc.sync.dma_start(xT_sb[:, m], xT.ap()[:, m])
                for ks_ in range(4, KO, 4):
                    nc.scalar.dma_start(wv_sb[:, ks_:ks_ + 4, :],
                                        wv.ap()[:, ks_:ks_ + 4, :])
                nc.scalar.dma_start(wk_sb, wk.ap())
                nc.scalar.dma_start(cos_sb, cosT.ap())
                nc.scalar.dma_start(sin_sb, sinT.ap())
                nc.scalar.dma_start(wq_sb, wq.ap())
                nc.scalar.dma_start(mask_sb, maskT.ap())
                nc.scalar.dma_start(wo_sb, wo.ap())
                # dummy exp so the ACT Exp table load (~1.3us on the ACT
                # sequencer) happens AFTER the Activation-queue descriptors
                # are generated -- issuing it earlier delays the weight
                # stream's first transfer
                nc.scalar.activation(warm, ones[0:1, 0:1],
                                     mybir.ActivationFunctionType.Exp)

                # attends for q-tile 0 run EMBEDDED in the Q-projection loop
                # below (see a0_steps); their tiles live in phase-1 pools
                a0_pools = (ps1, 2, ps1, 1, w_pool, 3, w_pool, 2, w_pool, 2)

                # warmup rhs: borrow an "ax" ring slot (zeroed; the ring
                # recycles it for attention later)
                warm_mm = w_pool.tile([P, QT], bf16, tag="ax", bufs=3,
                                      name="warm_mm")
                nc.vector.memset(warm_mm, 0.0)

                # PE p-state warmup: the tensor engine clocks up only after
                # ~3us of continuous execution, so chew on zeros while the
                # first wv/xT DMAs land -- the first real matmuls then run at
                # full clock instead of half.
                ps_w = ps1.tile([P, QT], fp32, tag="psv", bufs=4)
                for r in range(5):
                    nc.tensor.matmul(ps_w, ones, warm_mm,
                                     start=(r == 0), stop=(r == 4))

                # V: [keys, dg] natural layout, keychunk tiles of 128.
                # k-OUTER waves of m-groups: each wv k-slice unlocks one
                # matmul per group, so the PE ramps as the k-sliced wv DMA
                # trickles in instead of waiting for all of wv. The first
                # two waves are 2-group so they only gate on xT blocks 0-1
                # (resp. 2-3) while the stream is still ramping.
                for mw, nwv in ((0, 2), (2, 2), (4, 4), (8, 4), (12, 4)):
                    pss = [ps1.tile([P, DG], fp32, tag="psv",
                                    name=f"psv{i}", bufs=4)
                           for i in range(nwv)]
                    for k in range(KO):
                        for i in range(nwv):
                            nc.tensor.matmul(pss[i], xT_sb[:, mw + i, k, :],
                                             wv_sb[:, k, :],
                                             start=(k == 0),
                                             stop=(k == KO - 1))
                    for i in range(nwv):
                        nc.vector.tensor_copy(v_sb[:, mw + i, :], pss[i])

                # qt=0 attention, sliced into steps interleaved with the
                # Q-projection matmuls: each chunk's attn@V runs one step
                # AFTER its scores+exp, so the exp latency hides under the
                # projection matmuls between steps and the in-order PE never
                # stalls. Head 3 is held back (gate 99) to run AFTER the Q
                # loop, covering the last head-pair's serialized RoPE drain
                # (~4us of DVE) that otherwise gates the phase transition.
                a0_steps = []  # [(min_nt2, closure)]
                for h0 in range(NH):
                    st0 = {}
                    gate = 99 if h0 == NH - 1 else h0 // 2 + 1

                    def mk_sc(h0, m, st0):
                        return lambda: attend_scores(0, h0, m, st0, a0_pools)

                    def mk_av_sc(h0, m, st0):
                        def f():
                            attend_av(0, h0, m - 1, st0, a0_pools)
                            attend_scores(0, h0, m, st0, a0_pools)
                        return f

                    def mk_av(h0, m, st0):
                        return lambda: attend_av(0, h0, m, st0, a0_pools)

                    a0_steps.append((gate, mk_sc(h0, 0, st0)))
                    for m in range(1, 4):
                        a0_steps.append((gate, mk_av_sc(h0, m, st0)))
                    a0_steps.append((gate, mk_av(h0, 3, st0)))

                    def mk_fin(h0=h0, st0=st0):
                        def fin():
                            pending_fin.append(
                                (st0["total"], st0["ps_o"],
                                 ao0_sb[:, h0, :]))
                            flush_fin(ps1, w_pool, 1)
                        return fin
                    a0_steps.append((gate, mk_fin(h0, st0)))
                a0_next = 0

                # K then Q: [HD, S] transposed layout + RoPE.
                # Heads processed in pairs so the two psum tags can be
                # double-buffered (2 tags x 2 bufs) -- RoPE of one pair
                # overlaps the matmuls of the next.
                for which, w_sb, dstT in (("k", wk_sb, kT_sb),
                                          ("q", wq_sb, qT_sb)):
                    for nt2 in range(2 * NQT):
                        nt, hp = divmod(nt2, 2)
                        sl = slice(nt * QT, (nt + 1) * QT)
                        heads = (2 * hp, 2 * hp + 1)
                        # share the "psv" tag (banks 0-3) so the projections
                        # stay within 4 psum banks, leaving 4-7 free for the
                        # embedded attends
                        pss = {}
                        for h in heads:
                            pss[h] = ps1.tile([P, QT], fp32, tag="psv",
                                              name=f"psp{h}", bufs=4)
                        # rhs: the nt-th 512 queries = xT m-blocks 4nt..4nt+3
                        # at fixed k -- a strided [128, 4, 128] AP
                        for k in range(KO):
                            for h in heads:
                                nc.tensor.matmul(
                                    pss[h], w_sb[:, k, h * HD:(h + 1) * HD],
                                    xT_sb[:, nt * 4:(nt + 1) * 4, k, :],
                                    start=(k == 0), stop=(k == KO - 1))
                            if which == "q" and k % 4 == 3:
                                # attend step slot (only once the needed qT
                                # head has been roped: min_nt2 gate)
                                if (a0_next < len(a0_steps)
                                        and a0_steps[a0_next][0] <= nt2):
                                    a0_steps[a0_next][1]()
                                    a0_next += 1
                        for h in heads:
                            ps = pss[h]
                            dst = dstT[:, h, sl]
                            # rope: dst = ps * cos + swap(ps) * sin_signed.
                            # The swapped reads must come from PSUM (the SB-SB
                            # same-base-partition rule forbids them on SBUF);
                            # the straight read goes via a parallel ACT copy
                            # so the psum bank drains fast.
                            tmp = w_pool.tile([P, QT], bf16, tag="tmp",
                                              bufs=4, name="tmp")
                            nc.vector.tensor_mul(tmp[0:64], ps[64:128],
                                                 sin_sb[0:64, sl])
                            nc.vector.tensor_mul(tmp[64:128], ps[0:64],
                                                 sin_sb[64:128, sl])
                            qb = w_pool.tile([P, QT], bf16, tag="qb",
                                             bufs=4, name="qb")
                            nc.scalar.copy(qb, ps)
                            nc.vector.tensor_mul(dst, qb, cos_sb[:, sl])
                            nc.vector.tensor_add(dst, dst, tmp)

                # any attend steps not yet drained (shouldn't happen: 20
                # steps vs 28 slots)
                while a0_next < len(a0_steps):
                    a0_steps[a0_next][1]()
                    a0_next += 1

            # ---------------- phases 2+3 ----------------
            with tc.tile_pool(name="big2", bufs=1) as big2, \
                 tc.tile_pool(name="ps2", bufs=1, space="PSUM") as ps2:
                aoT_sb = big2.tile([P, NH, S], bf16)  # attention out^T
                # pre-create the psum tags whose first use comes late, so
                # the scores tag (first phase-2 PE writes) lands on the
                # banks the embedded attends freed earliest
                _d0 = ps2.tile([P, QT], fp32, tag="ps_out", bufs=2,
                               name="d0")
                _d1 = ps2.tile([P, QT], fp32, tag="ps_sum", bufs=1,
                               name="d1")
                a2_pools = (ps2, 3, ps2, 2, big2, 20, big2, 6, big2, 3)

                def emit_outproj(qo, split_dma=False, flush_mid=False):
                    # one [128, 2048] staging tile per query-row block: a
                    # single contiguous output DMA instead of four
                    # (descriptor generation is ~0.6us of sequencer time
                    # each). Casts alternate DVE/ACT except on emits 11-12,
                    # where ACT is still busy with the last attend's exps --
                    # those go DVE-only.
                    ob = big2.tile([P, D], bf16, tag="ob", bufs=2)
                    for n in range(D // QT):
                        nsl = slice(n * QT, (n + 1) * QT)
                        ps = ps2.tile([P, QT], fp32, tag="ps_out", bufs=2)
                        for h in range(NH):
                            if qo < 4:
                                lhs = ao0_sb[:, h, qo * P:(qo + 1) * P]
                            else:
                                lhs = aoT_sb[:, h, qo * P:(qo + 1) * P]
                            nc.tensor.matmul(
                                ps, lhs, wo_sb[:, h, nsl],
                                start=(h == 0), stop=(h == NH - 1))
                        if n == 2 and flush_mid:
                            # the last attend's finalize, emitted here so
                            # its DVE normalize queues ahead of this emit's
                            # remaining casts (the following emits read the
                            # normalized aoT)
                            flush_fin(ps2, big2, 2)
                        if n % 2 == 1 and (qo <= 10 or qo >= 13):
                            nc.scalar.copy(ob[:, nsl], ps)
                        else:
                            nc.vector.tensor_copy(ob[:, nsl], ps)
                        if split_dma:
                            # last row block: per-slice DMAs so the final
                            # transfer starts right after its own cast
                            nc.sync.dma_start(out.ap()[:, qo, nsl],
                                              ob[:, nsl])
                    if not split_dma:
                        nc.sync.dma_start(out.ap()[:, qo, :], ob)

                # attention for q-tiles 1-3, interleaved with the
                # out-projection rows the previous q-tile unblocked (those
                # matmuls have no ACT dependency and fill the exp-latency
                # bubbles). Each attend's finalize is flushed inside the
                # NEXT attend's chunk loop (pending_fin).
                for qt in range(1, NQT):
                    qsl = slice(qt * QT, (qt + 1) * QT)
                    for h in range(NH):
                        st = {}
                        for m in range(4 * (qt + 1)):
                            if m == 2:
                                flush_fin(ps2, big2, 2)
                            attend_chunk(qt, h, m, st, a2_pools)
                        pending_fin.append(
                            (st["total"], st["ps_o"], aoT_sb[:, h, qsl]))
                        emit_outproj(4 * (qt - 1) + h,
                                     flush_mid=(qt == NQT - 1
                                                and h == NH - 1))

                # the last q-tile's rows have nothing to hide behind
                for qo in range(4 * (NQT - 1), 4 * NQT):
                    emit_outproj(qo, split_dma=(qo == 4 * NQT - 1))

    nc.compile()
    return nc


def _rope_tables():
    inv_freq = 1.0 / (ROPE_THETA ** (np.arange(0, HD, 2, dtype=np.float64) / HD))
    pos = np.arange(S, dtype=np.float64)
    freqs = np.outer(pos, inv_freq)                    # [S, HD/2]
    emb = np.concatenate([freqs, freqs], axis=-1)      # [S, HD]
    cos = np.cos(emb).T.astype(BF16)                   # [HD, S]
    sin = np.sin(emb).T.astype(np.float32)
    sin[: HD // 2] *= -1.0                             # fold rotate_half sign
    return cos, sin.astype(BF16)


def _pack_kd(a):
    """[D, N] -> [P, D//P, N] with d = ko*P + p."""
    d, n = a.shape
    return np.ascontiguousarray(
        a.reshape(d // P, P, n).transpose(1, 0, 2)).astype(BF16)


def _pack_xT(xb):
    """x[b] [S, D] -> [P, NKT, KO, P] m-major so each 128-col block of x^T
    is one contiguous DMA."""
    t = _pack_kd(np.ascontiguousarray(xb.T))           # [P, KO, S]
    return np.ascontiguousarray(
        t.reshape(P, KO, NKT, P).transpose(0, 2, 1, 3))


def make_in_maps(x, wq, wk, wv, wo):
    cosT, sinT = _rope_tables()
    i = np.arange(P)[:, None]
    j = np.arange(P)[None, :]
    mask = (i <= j).astype(BF16)

    xT_packed = [_pack_xT(x[b]) for b in range(B)]
    in_maps = []
    for c in range(N_CORES):
        b, g = divmod(c, G)
        gsl = slice(g * DG, (g + 1) * DG)
        in_maps.append({
            "xT": xT_packed[b],
            "wq": _pack_kd(wq[:, gsl]),
            "wk": _pack_kd(wk[:, gsl]),
            "wv": _pack_kd(wv[:, gsl]),
            "wo": _pack_kd(np.ascontiguousarray(wo[gsl, :])),
            "cosT": cosT,
            "sinT": sinT,
            "maskT": mask,
        })
    return in_maps


def assemble_output(results):
    """results: list of 8 dicts with 'out' [P, NKT, D] bf16 partials."""
    full = np.empty((B, S, D), dtype=np.float32)
    for b in range(B):
        acc = None
        for g in range(G):
            r = results[b * G + g]["out"].astype(np.float32)
            part = r.transpose(1, 0, 2).reshape(S, D)
            acc = part if acc is None else acc + part
        full[b] = acc
    return full


def _get_module():
    global _BUILT
    if _BUILT is None:
        _BUILT = build_module()
    return _BUILT


def _install_trace_shim():
    """This image's antenv lacks axon_hooks; provide the NTFF profile hook
    via ctypes so trace=True (or BASS_TRACE=1) works instead of crashing,
    and skip the artifact bucket upload."""
    try:
        import antenv.axon_hooks  # noqa: F401
        return
    except ImportError:
        pass
    import types
    import ctypes
    import contextlib

    so_path = "/opt/axon/libaxon_pjrt.so"
    mod = types.ModuleType("antenv.axon_hooks")
    try:
        lib = ctypes.CDLL(so_path)
        lib.axon_start_nrt_profile.argtypes = [
            ctypes.POINTER(ctypes.c_int64), ctypes.c_size_t]
        lib.axon_start_nrt_profile.restype = ctypes.c_int64
        lib.axon_stop_nrt_profile.argtypes = [ctypes.c_char_p]
        lib.axon_stop_nrt_profile.restype = ctypes.c_int64

        @contextlib.contextmanager
        def _hook(output_dir, device_ids):
            import jax
            jax.devices()
            if device_ids:
                ids = (ctypes.c_int64 * len(device_ids))(*device_ids)
                rc = lib.axon_start_nrt_profile(ids, len(device_ids))
            else:
                rc = lib.axon_start_nrt_profile(None, 0)
            if rc != 0:
                raise RuntimeError(f"axon_start_nrt_profile rc={rc}")
            try:
                yield
            finally:
                lib.axon_stop_nrt_profile(str(output_dir).encode())

        mod.get_axon_ntff_profile_hook = lambda: _hook
    except OSError:
        mod.get_axon_ntff_profile_hook = lambda: None
    mod.set_axon_ntff_profile_hook = lambda h: None
    sys.modules["antenv.axon_hooks"] = mod

    from concourse import bass_utils
    bass_utils.upload_artifacts = lambda tmpdir: tmpdir


def run_on_hw(in_maps, trace=False, trace_cores=None):
    _install_trace_shim()
    from concourse import bass_utils
    nc = _get_module()
    return bass_utils.run_bass_kernel_spmd(
        nc, in_maps, core_ids=list(range(N_CORES)),
        trace=trace, trace_cores=trace_cores)


def kernel(x, wq, wk, wv, wo):
    x = np.asarray(x, dtype=np.float32)
    wq = np.asarray(wq, dtype=np.float32)
    wk = np.asarray(wk, dtype=np.float32)
    wv = np.asarray(wv, dtype=np.float32)
    wo = np.asarray(wo, dtype=np.float32)
    in_maps = make_in_maps(x, wq, wk, wv, wo)
    res = run_on_hw(in_maps, trace=False)
    return assemble_output(res.results)

